# revision 17
# baseline (speedup 1.0000x reference)
"""Multi-head attention (b=2, n=2048, dim=1024, 16 heads x 64) on 8 TRN2 NeuronCores.

Sharding: core c handles batch c//4 and heads 4*(c%4) .. 4*(c%4)+3
(data parallel over batch x 4-way head/tensor parallel). w_qkv is
column-sharded by head; w_out is column-sharded: each core computes a
256-column slice of the output after AllGathers of the attention outputs
within its 4-core batch group (no all-reduce needed).

Device layout is feature-major ("K-major"): x arrives pre-transposed
[dim, n] in bf16; Q^T/K^T are produced feature-major and V token-major
directly from the QKV projection (no on-chip transposes); attention
scores are computed transposed (dotsT[k, q]); softmax sums come from an
augmented ones-column in the V matmul; softmax exp runs on the scalar
engine with the 1/sqrt(d) scale folded in.

v2 schedule: QKV is projected per head-pair so attention for head pair 0
starts as early as possible; head pair 1's projections are injected into
tensor-engine gaps during the first attention blocks. The softmax
reciprocal runs on a DMA-transposed [128,4] layout (4 elems/partition
instead of 512 — the DVE divide is ~8 cyc/elem). AllGathers fire per
(head pair, q-block) right after each block normalizes, and the output
projection consumes each gather as it lands, so the tail after the last
exp is short. The final output is produced transposed [cols, n]; the
host transposes back.
"""

import sys

sys.path.insert(0, "/opt/trn_rl_repo")

import ml_dtypes
import numpy as np

import concourse.bass as bass  # noqa: F401  (engine types)
import concourse.tile as tile
from concourse import bacc, mybir
from concourse.bass_utils import run_bass_kernel_spmd

F32 = mybir.dt.float32
F32R = mybir.dt.float32r
BF16 = mybir.dt.bfloat16
NP_BF16 = np.dtype(ml_dtypes.bfloat16)

# Problem constants
B, N, DIM = 2, 2048, 1024
HEADS, DH = 16, 64
INNER = HEADS * DH
SCALE = DH ** -0.5
CORES = 8
GROUP_SIZE = 4
REPLICA_GROUPS = [[0, 1, 2, 3], [4, 5, 6, 7]]
HPC = 4  # heads per core
CS = HPC * DH  # 256 per-core feature columns

KC = DIM // 128  # 8 contraction chunks for dim
TT = N // 128  # 16 token tiles
QB = N // 512  # 4 q blocks
NKC = N // 128  # 16 key chunks
NBLK = 2 * QB  # 8 attention blocks (head pair x q block)


def build_nc():
    nc = bacc.Bacc("TRN2", target_bir_lowering=False, debug=False, num_devices=CORES)
    xt = nc.dram_tensor("xt", [DIM, N], BF16, kind="ExternalInput").ap()
    wq = nc.dram_tensor("wq", [DIM, CS], BF16, kind="ExternalInput").ap()
    wk = nc.dram_tensor("wk", [DIM, CS], BF16, kind="ExternalInput").ap()
    wv = nc.dram_tensor("wv", [DIM, CS], BF16, kind="ExternalInput").ap()
    wo = nc.dram_tensor("wo", [INNER, CS], BF16, kind="ExternalInput").ap()
    bo = nc.dram_tensor("bo", [CS], F32, kind="ExternalInput").ap()
    y = nc.dram_tensor("y", [CS, N], F32, kind="ExternalOutput").ap()  # y^T

    # per-(head pair, q block) collective staging
    cc_in = [
        [nc.dram_tensor(f"cc_in{m}_{q}", [128, 512], BF16) for q in range(QB)]
        for m in range(2)
    ]
    cc_out = [
        [
            nc.dram_tensor(f"cc_out{m}_{q}", [GROUP_SIZE * 128, 512], BF16)
            for q in range(QB)
        ]
        for m in range(2)
    ]

    with tile.TileContext(nc) as tc:
        with (
            tc.tile_pool(name="big", bufs=2) as big,  # xt + AG landing area
            tc.tile_pool(name="sb", bufs=1) as sb,
            tc.tile_pool(name="expp", bufs=4) as expp,
            tc.tile_pool(name="yout", bufs=3) as yout,
            tc.tile_pool(name="norm", bufs=4) as normp,
            tc.tile_pool(name="psd", bufs=2, space="PSUM") as psd,
            tc.tile_pool(name="pso", bufs=2, space="PSUM") as pso,
            tc.tile_pool(name="psy", bufs=2, space="PSUM") as psyp,
        ):
            # ---- warm up the collective stream ------------------------------
            # the first collective op pays a ~35us all-core rendezvous
            # barrier; trigger it immediately so the real AllGathers aren't
            # gated on it.
            ccw_in = nc.dram_tensor("ccw_in", [1, 64], BF16)
            ccw_out = nc.dram_tensor("ccw_out", [GROUP_SIZE, 64], BF16)
            nc.gpsimd.collective_compute(
                "AllGather",
                mybir.AluOpType.bypass,
                ins=[ccw_in.ap().opt()],
                outs=[ccw_out.ap().opt()],
                replica_groups=REPLICA_GROUPS,
            )

            # ---- load inputs -------------------------------------------------
            # q block 0's inputs first so attention block 0 starts asap
            xt_sb = big.tile([128, KC, N], BF16, tag="bigbuf")
            wq_sb = sb.tile([128, KC, CS], BF16)
            wk_sb = sb.tile([128, KC, CS], BF16)
            wv_sb = sb.tile([128, KC, CS], BF16)
            wo_sb = sb.tile([128, KC, CS], BF16)
            wk_r = wk.rearrange("(c p) n -> p c n", p=128)
            wq_r = wq.rearrange("(c p) n -> p c n", p=128)
            xt_r = xt.rearrange("(c p) n -> p c n", p=128)
            for c in range(KC):
                nc.sync.dma_start(out=xt_sb[:, c, 0:512], in_=xt_r[:, c, 0:512])
            nc.sync.dma_start(out=wk_sb[:, :, 0:128], in_=wk_r[:, :, 0:128])
            nc.sync.dma_start(out=wq_sb[:, :, 0:128], in_=wq_r[:, :, 0:128])
            nc.sync.dma_start(out=wv_sb, in_=wv.rearrange("(c p) n -> p c n", p=128))
            for qb in range(1, QB):
                sl = slice(qb * 512, (qb + 1) * 512)
                for c in range(KC):
                    nc.sync.dma_start(out=xt_sb[:, c, sl], in_=xt_r[:, c, sl])
            nc.sync.dma_start(out=wk_sb[:, :, 128:256], in_=wk_r[:, :, 128:256])
            nc.sync.dma_start(out=wq_sb[:, :, 128:256], in_=wq_r[:, :, 128:256])
            nc.sync.dma_start(out=wo_sb, in_=wo.rearrange("(c p) n -> p c n", p=128))

            # bias, transposed layout: partition = column-within-block
            bias_sb = sb.tile([128, 2], F32)
            nc.sync.dma_start(out=bias_sb, in_=bo.rearrange("(cb p) -> p cb", p=128))

            ones_f = sb.tile([128, TT], F32)
            nc.vector.memset(ones_f, 1.0)

            qt_sb = sb.tile([128, 2, N], BF16)
            kt_sb = sb.tile([128, 2, N], BF16)
            vaug = sb.tile([128, TT, HPC, DH + 1], BF16)
            with nc.allow_low_precision(reason="bf16 ones column"):
                for h in range(HPC):
                    nc.vector.tensor_copy(vaug[:, :, h, DH], ones_f)

            # ---- QKV building blocks ----------------------------------------
            # All QKV groups allocate from the psy pool so they don't perturb
            # the dots double-buffer in psd.
            def qkv_group(m, qb, dst, w_sb, pool):
                acc = pool.tile([128, 512], F32, name="psy")
                for c in range(KC):
                    nc.tensor.matmul(
                        acc,
                        lhsT=w_sb[:, c, m * 128 : (m + 1) * 128],
                        rhs=xt_sb[:, c, qb * 512 : (qb + 1) * 512],
                        start=(c == 0),
                        stop=(c == KC - 1),
                    )
                with nc.allow_low_precision(reason="bf16 attention"):
                    nc.vector.tensor_copy(dst[:, m, qb * 512 : (qb + 1) * 512], acc)

            def v_tile(t, pool):
                # one pass over all 4 heads: the v matmuls are weight-load
                # bound, so free=256 costs the same as two free=128 passes
                ps = pool.tile([128, 512], F32, name="psy")
                acc = ps[:, 0:CS]
                for c in range(KC):
                    nc.tensor.matmul(
                        acc,
                        lhsT=xt_sb[:, c, t * 128 : (t + 1) * 128],
                        rhs=wv_sb[:, c, :],
                        start=(c == 0),
                        stop=(c == KC - 1),
                    )
                with nc.allow_low_precision(reason="bf16 attention"):
                    nc.vector.tensor_copy(
                        vaug[:, t, :, 0:DH],
                        acc.rearrange("p (h d) -> p h d", d=DH),
                    )

            # ---- attention building blocks ----------------------------------
            outt_sb = sb.tile([128, 2, N], BF16)
            ag_all = big.tile([128, 2, QB, GROUP_SIZE, 512], BF16, tag="bigbuf")
            y_acc = sb.tile([128, 2, N], F32)

            def emit_dots(blk, kc):
                hp, qb = divmod(blk, QB)
                ps = psd.tile([128, 2, 512], F32, name="psd")
                for hh in range(2):
                    base = hh * DH
                    nc.tensor.matmul(
                        ps[:, hh, :],
                        lhsT=kt_sb[base : base + DH, hp, kc * 128 : (kc + 1) * 128],
                        rhs=qt_sb[base : base + DH, hp, qb * 512 : (qb + 1) * 512],
                        start=True,
                        stop=True,
                        tile_position=(base, 0),
                    )
                ex = expp.tile([128, 2, 512], BF16, name="expT")
                nc.scalar.activation(
                    out=ex, in_=ps, func=mybir.ActivationFunctionType.Exp, scale=SCALE
                )
                return ex

            def emit_attv(blk, kc, ex, po):
                hp = blk // QB
                for hh in range(2):
                    nc.tensor.matmul(
                        po[hh],
                        lhsT=vaug[:, kc, hp * 2 + hh, :],
                        rhs=ex[:, hh, :],
                        start=(kc == 0),
                        stop=(kc == NKC - 1),
                    )

            def emit_recip_chain(po_pair):
                """Copy po to SBUF; compute 1/Z on a DMA-reshaped [128, 4]
                layout (the DVE divide is ~8 cyc per free-dim element, so
                spreading the 512 Z values across partitions makes it ~64x
                cheaper than on [*, 512])."""
                out = []
                for hh in range(2):
                    po_sb = normp.tile([DH + 1, 512], F32, name="po_sb")
                    nc.vector.tensor_copy(po_sb, po_pair[hh])
                    zt = normp.tile([128, 4], F32, name="zT")
                    nc.sync.dma_start(out=zt, in_=po_sb[DH : DH + 1, :])
                    nc.vector.reciprocal(zt, zt)
                    zinv_row = normp.tile([1, 512], F32, name="zinv")
                    nc.sync.dma_start(out=zinv_row, in_=zt)
                    # broadcast 1/Z across the 64 head dims with a DMA
                    # (free-dim stride-0 source), not a PE matmul
                    zb = normp.tile([DH, 512], F32, name="zb")
                    nc.sync.dma_start(
                        out=zb,
                        in_=zinv_row.rearrange("p (o f) -> p o f", o=1).broadcast_to(
                            [1, DH, 512]
                        ),
                    )
                    out.append((po_sb, zb))
                return out

            def emit_norm(blk, pairs):
                hp, qb = divmod(blk, QB)
                for hh, (po_sb, zb) in enumerate(pairs):
                    base = hh * DH
                    with nc.allow_low_precision(reason="bf16 attention out"):
                        nc.vector.tensor_mul(
                            outt_sb[base : base + DH, hp, qb * 512 : (qb + 1) * 512],
                            po_sb[0:DH, :],
                            zb,
                        )

            def emit_ag(blk):
                hp, qb = divmod(blk, QB)
                sl = slice(qb * 512, (qb + 1) * 512)
                nc.gpsimd.dma_start(out=cc_in[hp][qb].ap(), in_=outt_sb[:, hp, sl])
                nc.gpsimd.collective_compute(
                    "AllGather",
                    mybir.AluOpType.bypass,
                    ins=[cc_in[hp][qb].ap().opt()],
                    outs=[cc_out[hp][qb].ap().opt()],
                    replica_groups=REPLICA_GROUPS,
                )
                nc.sync.dma_start(
                    out=ag_all[:, hp, qb, :, :],
                    in_=cc_out[hp][qb].ap().rearrange("(c p) n -> p c n", p=128),
                )

            def emit_outproj(hp, qb, cb):
                """One [128, 512] slab of y^T from head-pair hp's gathered
                attention output for q block qb."""
                ps = psyp.tile([128, 512], F32, name="psy")
                for c in range(4):
                    nc.tensor.matmul(
                        ps,
                        lhsT=wo_sb[:, hp * 4 + c, cb * 128 : (cb + 1) * 128],
                        rhs=ag_all[:, hp, qb, c, :],
                        start=(c == 0),
                        stop=(c == 3),
                    )
                ysl = slice(qb * 512, (qb + 1) * 512)
                if hp == 0:
                    # fold the bias into pass 1
                    nc.vector.tensor_scalar_add(
                        out=y_acc[:, cb, ysl], in0=ps, scalar1=bias_sb[:, cb : cb + 1]
                    )
                else:
                    y_sb = yout.tile([128, 512], F32, name="y_sb")
                    nc.vector.tensor_add(y_sb, ps, y_acc[:, cb, ysl])
                    nc.sync.dma_start(
                        out=y[cb * 128 : (cb + 1) * 128, ysl], in_=y_sb
                    )

            # ---- prologue: just enough for attention block 0 to start -------
            qkv_group(0, 0, kt_sb, wk_sb, psyp)
            qkv_group(0, 0, qt_sb, wq_sb, psyp)
            for t in range(4):
                v_tile(t, psyp)

            # ---- injected work schedule -------------------------------------
            # remaining projections are emitted just ahead of their first
            # consumer: k m0 qb j before dots(0, 4j); v tile t before
            # attV(0, t) (emitted at step (0, t+1)); q m0 qb j before block j;
            # head-pair 1 k/q spread over blocks 1-3. Out-projection chunks
            # run well after their AllGather was issued.
            schedule = {}

            def put(key, fn):
                schedule.setdefault(key, []).append(fn)

            for j in range(1, QB):
                put((0, 4 * j - 3), (lambda q: lambda: qkv_group(0, q, kt_sb, wk_sb, psyp))(j))
            for t in range(4, TT):
                put((0, t), (lambda tt: lambda: v_tile(tt, psyp))(t))
            put((0, 13), lambda: qkv_group(0, 1, qt_sb, wq_sb, psyp))
            put((1, 1), lambda: qkv_group(0, 2, qt_sb, wq_sb, psyp))
            put((2, 1), lambda: qkv_group(0, 3, qt_sb, wq_sb, psyp))
            # k m1 qb j: needed by dots(4, 4j); q m1 qb j: by dots(4 + j, 0)
            for j in range(QB):
                put((1, 4 * j + 3), (lambda q: lambda: qkv_group(1, q, kt_sb, wk_sb, psyp))(j))
            put((2, 5), lambda: qkv_group(1, 0, qt_sb, wq_sb, psyp))
            put((2, 11), lambda: qkv_group(1, 1, qt_sb, wq_sb, psyp))
            put((3, 3), lambda: qkv_group(1, 2, qt_sb, wq_sb, psyp))
            put((3, 11), lambda: qkv_group(1, 3, qt_sb, wq_sb, psyp))
            # out-proj: AG(blk) fires at (blk+1, 6); schedule chunks well
            # after (each AG takes ~15us). Blocks 4-7 drain after the loop.
            for blk in range(4):
                hp, qb = divmod(blk, QB)
                put((blk + 4, 4), (lambda h, q: lambda: emit_outproj(h, q, 0))(hp, qb))
                put((blk + 4, 12), (lambda h, q: lambda: emit_outproj(h, q, 1))(hp, qb))

            # ---- attention: software-pipelined over all 8 blocks ------------
            pend_attv = None  # (blk, kc, ex)
            pend_recip = None  # (blk, pairs) awaiting norm
            po_cur = None
            po_prev = None
            for blk in range(NBLK):
                for kc in range(NKC):
                    if kc == 0:
                        po_prev = po_cur
                        po_cur = [
                            pso.tile([DH + 1, 512], F32, name="ps_o") for _ in range(2)
                        ]
                    for fn in schedule.get((blk, kc), ()):
                        fn()
                    ex = emit_dots(blk, kc)
                    if pend_attv is not None:
                        pblk, pkc, pex = pend_attv
                        emit_attv(pblk, pkc, pex, po_cur if pblk == blk else po_prev)
                        if pkc == NKC - 1:
                            pend_recip2 = (pblk, emit_recip_chain(po_prev))
                    pend_attv = (blk, kc, ex)
                    if kc == 6 and blk > 0:
                        nblk, pairs = pend_recip
                        emit_norm(nblk, pairs)
                        emit_ag(nblk)
                    if kc == 0 and blk > 0:
                        pend_recip = pend_recip2
            # drain the pipeline (block 6's norm/AG already ran at (7, 6))
            pblk, pkc, pex = pend_attv
            emit_attv(pblk, pkc, pex, po_cur)
            pairs7 = emit_recip_chain(po_cur)
            # blocks 4-5's out-proj overlaps block 7's 1/Z DMA round trip
            for blk in (4, 5):
                hp, qb = divmod(blk, QB)
                emit_outproj(hp, qb, 0)
                emit_outproj(hp, qb, 1)
            emit_norm(NBLK - 1, pairs7)
            emit_ag(NBLK - 1)
            for blk in (6, 7):
                hp, qb = divmod(blk, QB)
                emit_outproj(hp, qb, 0)
                emit_outproj(hp, qb, 1)

    nc.compile()
    return nc


_NC_CACHE = None


def _get_nc():
    global _NC_CACHE
    if _NC_CACHE is None:
        _NC_CACHE = build_nc()
    return _NC_CACHE


def _wo_perm(w_out):
    # chunk order [AG0: r0..r3 -> w_out rows 256r..256r+128,
    #              AG1: r0..r3 -> w_out rows 256r+128..256r+256]
    blocks = [w_out[256 * r : 256 * r + 128] for r in range(4)]
    blocks += [w_out[256 * r + 128 : 256 * r + 256] for r in range(4)]
    return np.concatenate(blocks, axis=0)


def _make_in_maps(x, w_qkv, w_out, b_out):
    wop = _wo_perm(w_out)
    in_maps = []
    for c in range(CORES):
        bi = c // GROUP_SIZE
        g = c % GROUP_SIZE
        cols = slice(g * CS, (g + 1) * CS)
        in_maps.append(
            {
                "xt": np.ascontiguousarray(x[bi].T).astype(NP_BF16),
                "wq": np.ascontiguousarray(w_qkv[:, cols]).astype(NP_BF16),
                "wk": np.ascontiguousarray(w_qkv[:, INNER:][:, cols]).astype(NP_BF16),
                "wv": np.ascontiguousarray(w_qkv[:, 2 * INNER:][:, cols]).astype(
                    NP_BF16
                ),
                "wo": np.ascontiguousarray(wop[:, cols]).astype(NP_BF16),
                "bo": np.ascontiguousarray(b_out[cols]),
            }
        )
    return in_maps


def _assemble(results):
    out = np.empty((B, N, DIM), dtype=np.float32)
    for c in range(CORES):
        bi = c // GROUP_SIZE
        g = c % GROUP_SIZE
        out[bi, :, g * CS : (g + 1) * CS] = results[c]["y"].T
    return out


def kernel(x, w_qkv, w_out, b_out, _trace=False, _trace_kwargs=None):
    x = np.asarray(x, dtype=np.float32)
    w_qkv = np.asarray(w_qkv, dtype=np.float32)
    w_out = np.asarray(w_out, dtype=np.float32)
    b_out = np.asarray(b_out, dtype=np.float32)
    nc = _get_nc()
    in_maps = _make_in_maps(x, w_qkv, w_out, b_out)
    res = run_bass_kernel_spmd(
        nc,
        in_maps,
        core_ids=list(range(CORES)),
        trace=_trace,
        **(_trace_kwargs or {}),
    )
    out = _assemble(res.results)
    if _trace:
        return out, res
    return out
